# revision 63
# baseline (speedup 1.0000x reference)
"""Trainium2 Bass kernel for NeuralGraphHidden (GNN message passing).

Math (per molecule b, atom a):
    deg[b,a]    = #valid edges (edges[b,a,:] != -1)
    summed_atom = atoms[b,a] + sum_s atoms[b, edges[b,a,s]]          (64)
    bond_sum    = sum_s bonds[b,a,s]                                  (8)
    x           = concat(summed_atom, bond_sum)                      (72)
    out[b,a]    = relu(x @ Ws[deg] + bs[deg])  if deg <= 5 else 0   (128)

Design notes (driven by measured TRN2 behaviour on this system):
  * Device-side random-row gathers measured 20-500 ns/row -> the host does
    all *layout* work (degree-sort permutation, neighbour row expansion via
    np.take, bf16 packing, feature-major transposes), which is pure indexed
    data movement; the device does all arithmetic.
  * Everything is delivered FEATURE-MAJOR (partition = feature, free =
    degree-sorted token slot), so the device needs no transposes:
      - xrowsT [112, 15360]: rows 0:64 self atom features, rows 64:112 the
        six raw bond vectors; the bond sum happens inside the matmul because
        Wb is tiled 6x along K in wpack.
      - npairT [128, 23040]: neighbour atom features packed TWO SLOTS PER
        COLUMN (s=2p in partitions 0:64, s=2p+1 in 64:128, zeros when
        2p+1 >= d).  A K=128 matmul against vertically stacked [Wa_d; Wa_d]
        sums both neighbour slots in one pass.  128-partition descriptors
        also load at full DMA rate - 64-partition tiles measured half rate.
      - out[c, tok] = relu(Wd^T x + b) with conv on PARTITIONS, so the bias
        is a per-partition scalar folded into the Scalar-engine relu.
  * Per degree group d (2560 slots), per 512-col quad (one PSUM bank): one
    K=112 main matmul + ceil(d/2) K=128 neighbour-pair matmuls accumulate
    in PSUM; Scalar engine applies bias+relu into bf16.  ~110 instructions.
  * DMA is the roofline (~13.4 MB/core).  ALL transfers ride ONE ring
    (sync) issued in consumption order - a single ring drains FIFO at full
    aggregate bandwidth and preserves arrival order; multi-ring issue
    measured ~40% slower.  Stores interleave after each group.
  * Host unpermutes the sorted output (deg-6 rows are zero).
"""

import sys

sys.path.insert(0, "/opt/trn_rl_repo")

import numpy as np
import ml_dtypes

from contextlib import ExitStack

import concourse.bacc as bacc
import concourse.tile as tile
from concourse import mybir
from concourse.bass_utils import run_bass_kernel_spmd

# Problem shapes (hardcoded per the harness contract).
B, A, D = 1024, 128, 6
F_ATOM, F_BOND, CONV = 64, 8, 128
NCORES = 8
BS = B // NCORES          # molecules per core = 128
T = BS * A                # tokens per core = 16384
ROW = F_ATOM + D * F_BOND               # 112 features per packed row
ROWP = 128                              # padded to 128 partitions: non-128-
                                        # partition DMAs measured ~0.6x rate
NPAIR = [(d + 1) // 2 for d in range(D)]    # neighbour s-pairs per group
DORDER = [0, 5, 4, 3, 2, 1]             # d0 while later loads stream; d1 tail


class Geom:
    """Slot geometry fitted to the actual input: group_pad is the max
    degree-group population over all cores, rounded up so each of the nq
    quads is an even number of columns (<= 512, one PSUM bank of f32)."""

    def __init__(self, max_count):
        self.nq = -(-max_count // 500)          # quads per group
        self.group_pad = -(-max_count // (2 * self.nq)) * (2 * self.nq)
        self.qw = self.group_pad // self.nq     # <= 502
        self.nsort = D * self.group_pad
        self.pcol = {}
        off = 0
        for d in (5, 4, 3, 2, 1):
            self.pcol[d] = off
            off += NPAIR[d] * self.group_pad
        self.npair_cols = off
# stores: paired for the early groups (10 KB descriptors move ~30% faster
# than 5 KB), single for the last three so results drain as soon as each
# group finishes (the ring was measured starving ~2 us waiting on d2/d1)
STORES = {0: (0, 1), 4: (4, 6), 3: (3, 4), 2: (2, 3), 1: (1, 2)}
# one combined bf16 constants tensor: [wpack | wstack | bias]
WC_PACK, WC_STACK, WC_BIAS = 0, D * CONV, 2 * D * CONV
WC_COLS = 2 * D * CONV + D              # 1542

_f32 = mybir.dt.float32
_bf16 = mybir.dt.bfloat16

_cached = {}


def build_program(g):
    """Build the per-core Bass/Tile program for slot geometry `g`."""
    nc = bacc.Bacc("TRN2", target_bir_lowering=False, debug=False)

    GROUP_PAD, QW = g.group_pad, g.qw
    xrowsT = nc.dram_tensor("xrowsT", [ROWP, g.nsort], _bf16,
                            kind="ExternalInput")
    npairT = nc.dram_tensor("npairT", [2 * F_ATOM, g.npair_cols], _bf16,
                            kind="ExternalInput")
    wcombo = nc.dram_tensor("wcombo", [ROWP, WC_COLS], _bf16,
                            kind="ExternalInput")
    osortT = nc.dram_tensor("osortT", [CONV, g.nsort], _bf16,
                            kind="ExternalOutput")

    with tile.TileContext(nc) as tc, ExitStack() as ctx:
        const_pool = ctx.enter_context(tc.tile_pool(name="const", bufs=1))
        work_pool = ctx.enter_context(tc.tile_pool(name="work", bufs=1))
        ps_pool = ctx.enter_context(tc.tile_pool(name="ps", bufs=8,
                                                 space="PSUM"))

        wc_t = const_pool.tile([ROWP, WC_COLS], _bf16, tag="wcombo")
        zt = const_pool.tile([CONV, 512], _bf16, tag="zt")
        nc.vector.memset(zt[:], 0.0)
        xall = work_pool.tile([ROWP, g.nsort], _bf16, tag="xall")
        np_t = {}
        for d in range(1, D):
            np_t[d] = work_pool.tile([2 * F_ATOM, NPAIR[d] * GROUP_PAD],
                                     _bf16, tag=f"np{d}", name=f"np{d}")
        out_t = {}
        for od, (c0, c1) in STORES.items():
            out_t[od] = work_pool.tile([CONV, (c1 - c0) * GROUP_PAD], _bf16,
                                       tag=f"out{od}", name=f"out{od}")

        # ONE ring (sync), consumption order.
        nc.sync.dma_start(out=wc_t[:], in_=wcombo[:])
        nc.sync.dma_start(out=xall[:], in_=xrowsT[:])
        for d in (5, 4, 3, 2, 1):
            nc.sync.dma_start(
                out=np_t[d][:],
                in_=npairT[:, g.pcol[d]:g.pcol[d] + NPAIR[d] * GROUP_PAD])

        def out_ap(d):
            od = next(o for o, (c0, c1) in STORES.items() if c0 <= d < c1)
            return out_t[od], od, (d - STORES[od][0]) * GROUP_PAD

        for d in DORDER:
            xt = xall[:, d * GROUP_PAD:(d + 1) * GROUP_PAD]
            out_g, od, ocol = out_ap(d)
            wmain = wc_t[:, WC_PACK + d * CONV:WC_PACK + (d + 1) * CONV]
            wpair = wc_t[:, WC_STACK + d * CONV:WC_STACK + (d + 1) * CONV]
            # per-quad PSUM tiles padded to a full 512-col bank so each
            # matmul accumulation group stays inside one bank even when
            # QW < 512 (PE writes cannot cross banks)
            for q in range(g.nq):
                cols = slice(q * QW, (q + 1) * QW)
                ps = ps_pool.tile([CONV, 512], _f32, tag="ps", name="ps")
                nc.tensor.matmul(out=ps[:, 0:QW], lhsT=wmain,
                                 rhs=xt[:, cols],
                                 start=True, stop=(NPAIR[d] == 0))
                for p in range(NPAIR[d]):
                    nc.tensor.matmul(
                        out=ps[:, 0:QW], lhsT=wpair,
                        rhs=np_t[d][:, p * GROUP_PAD + cols.start:
                                    p * GROUP_PAD + cols.stop],
                        start=False, stop=(p == NPAIR[d] - 1))
                # the last two groups gate the drain tail; alternate their
                # relu quads onto the otherwise-idle Vector engine so the
                # Scalar queue isn't the serial bottleneck at the end
                if d in (2, 1) and q % 2 == 1:
                    nc.vector.scalar_tensor_tensor(
                        out_g[:, ocol + cols.start:ocol + cols.stop],
                        ps[:, 0:QW],
                        wc_t[:, WC_BIAS + d:WC_BIAS + d + 1],
                        zt[:, 0:QW],
                        mybir.AluOpType.add,
                        mybir.AluOpType.max)
                else:
                    nc.scalar.activation(
                        out_g[:, ocol + cols.start:ocol + cols.stop],
                        ps[:, 0:QW],
                        mybir.ActivationFunctionType.Relu,
                        bias=wc_t[:, WC_BIAS + d:WC_BIAS + d + 1])
            if d == od:
                c0, c1 = STORES[od]
                nc.sync.dma_start(
                    out=osortT[:, c0 * GROUP_PAD:c1 * GROUP_PAD],
                    in_=out_t[od][:])

    nc.compile()
    return nc


def _get_program(g):
    if g.group_pad not in _cached:
        _cached[g.group_pad] = build_program(g)
    return _cached[g.group_pad]


def prep_core_inputs(atoms_s, bonds_s, edges_s, wcombo_np, g):
    """Host-side layout/index prep for one core's shard (numpy only)."""
    GROUP_PAD = g.group_pad
    deg = (edges_s != -1).sum(axis=-1).reshape(-1)            # [T] natural
    slot_tok = np.full(g.nsort, -1, np.int64)  # sorted slot -> natural token
    for d in range(D):
        toks = np.nonzero(deg == d)[0]
        n = len(toks)
        assert n <= GROUP_PAD, f"degree-{d} group has {n} > {GROUP_PAD}"
        slot_tok[d * GROUP_PAD:d * GROUP_PAD + n] = toks

    flat = np.concatenate(
        [atoms_s.reshape(T, F_ATOM), bonds_s.reshape(T, D * F_BOND)], axis=1
    ).astype(ml_dtypes.bfloat16)                              # [T, 112]
    safe = np.maximum(slot_tok, 0)
    xrows = np.where((slot_tok >= 0)[:, None], flat[safe],
                     ml_dtypes.bfloat16(0))                   # [nsort, 112]
    xrowsT = np.zeros((ROWP, g.nsort), ml_dtypes.bfloat16)
    xrowsT[:ROW] = xrows.T                                    # [128, nsort]

    eflat = edges_s.reshape(T, D)
    bcol = (np.arange(T) // A) * A                            # molecule base
    atoms_flat = flat[:, :F_ATOM]

    def neigh_rows(d, s):
        slots = slot_tok[d * GROUP_PAD:(d + 1) * GROUP_PAD]
        sv = slots >= 0
        st = np.maximum(slots, 0)
        e = np.where(sv, eflat[st, s], -1)
        nat = np.maximum(bcol[st] + e, 0)
        return np.where((e >= 0)[:, None], atoms_flat[nat],
                        ml_dtypes.bfloat16(0))                # [GROUP_PAD, 64]

    zero = np.zeros((GROUP_PAD, F_ATOM), ml_dtypes.bfloat16)
    regions = []
    for d in (5, 4, 3, 2, 1):                 # chunk order = load order
        for p in range(NPAIR[d]):
            lo = neigh_rows(d, 2 * p)
            hi = neigh_rows(d, 2 * p + 1) if 2 * p + 1 < d else zero
            regions.append(np.concatenate([lo, hi], axis=1))  # [GP, 128]
    npair = np.concatenate(regions, axis=0)
    npairT = np.ascontiguousarray(npair.T)                    # [128, ncols]

    return {
        "xrowsT": xrowsT,
        "npairT": npairT,
        "wcombo": wcombo_np,
    }, slot_tok


def kernel(atoms, bonds, edges, Ws, bs, trace=False):
    atoms = np.asarray(atoms)
    bonds = np.asarray(bonds)
    edges = np.asarray(edges)
    Ws = np.asarray(Ws)
    bs = np.asarray(bs)

    # wcombo = [wpack | wstack | biasT], all bf16, one DMA:
    #   wpack[:, d*128:(d+1)*128] = [Wa_d (64) | tile(Wb_d, 6) (48)]; the 6x
    #   tiling makes the matmul itself perform the bond sum.
    #   wstack[:, d*128:(d+1)*128] = [Wa_d; Wa_d] so a K=128 matmul sums a
    #   neighbour-slot pair in one pass.
    wcombo_np = np.zeros((ROWP, WC_COLS), np.float32)
    wfull = np.zeros((D, ROWP, CONV), np.float32)
    wfull[:, :F_ATOM] = Ws[:, :F_ATOM]
    wfull[:, F_ATOM:ROW] = np.tile(Ws[:, F_ATOM:], (1, D, 1))
    wcombo_np[:, WC_PACK:WC_STACK] = wfull.transpose(1, 0, 2).reshape(
        ROWP, D * CONV)
    wstack = np.concatenate([Ws[:, :F_ATOM], Ws[:, :F_ATOM]], axis=1)
    wcombo_np[:, WC_STACK:WC_BIAS] = wstack.transpose(1, 0, 2).reshape(
        2 * F_ATOM, D * CONV)
    wcombo_np[:, WC_BIAS:] = bs.T
    wcombo_np = np.ascontiguousarray(wcombo_np.astype(ml_dtypes.bfloat16))

    # fit the slot geometry to this input's actual degree populations
    deg_all = (edges != -1).sum(axis=-1).reshape(NCORES, T)
    maxc = 0
    for c in range(NCORES):
        maxc = max(maxc, int(np.bincount(deg_all[c], minlength=7)[:D].max()))
    g = Geom(maxc)

    in_maps, slot_toks = [], []
    for c in range(NCORES):
        sl = slice(c * BS, (c + 1) * BS)
        m, st = prep_core_inputs(atoms[sl], bonds[sl], edges[sl],
                                 wcombo_np, g)
        in_maps.append(m)
        slot_toks.append(st)

    nc = _get_program(g)
    res = run_bass_kernel_spmd(nc, in_maps, core_ids=list(range(NCORES)),
                               trace=trace)
    kernel.last_results = res

    out = np.zeros((B, A, CONV), np.float32)
    for c in range(NCORES):
        osortT = res.results[c]["osortT"].view(ml_dtypes.bfloat16)
        osort = osortT.reshape(CONV, g.nsort).T               # [nsort, 128]
        st = slot_toks[c]
        real = st >= 0
        shard = out[c * BS:(c + 1) * BS].reshape(T, CONV)
        shard[st[real]] = osort[real].astype(np.float32)
    return out


# revision 66
# speedup vs baseline: 1.0440x; 1.0440x over previous
"""Trainium2 Bass kernel for NeuralGraphHidden (GNN message passing).

Math (per molecule b, atom a):
    deg[b,a]    = #valid edges (edges[b,a,:] != -1)
    summed_atom = atoms[b,a] + sum_s atoms[b, edges[b,a,s]]          (64)
    bond_sum    = sum_s bonds[b,a,s]                                  (8)
    x           = concat(summed_atom, bond_sum)                      (72)
    out[b,a]    = relu(x @ Ws[deg] + bs[deg])  if deg <= 5 else 0   (128)

Design notes (driven by measured TRN2 behaviour on this system):
  * Device-side random-row gathers measured 20-500 ns/row -> the host does
    all *layout* work (degree-sort permutation, neighbour row expansion via
    np.take, bf16 packing, feature-major transposes), which is pure indexed
    data movement; the device does all arithmetic.
  * Everything is delivered FEATURE-MAJOR (partition = feature, free =
    degree-sorted token slot), so the device needs no transposes:
      - xrowsT [112, 15360]: rows 0:64 self atom features, rows 64:112 the
        six raw bond vectors; the bond sum happens inside the matmul because
        Wb is tiled 6x along K in wpack.
      - npairT [128, 23040]: neighbour atom features packed TWO SLOTS PER
        COLUMN (s=2p in partitions 0:64, s=2p+1 in 64:128, zeros when
        2p+1 >= d).  A K=128 matmul against vertically stacked [Wa_d; Wa_d]
        sums both neighbour slots in one pass.  128-partition descriptors
        also load at full DMA rate - 64-partition tiles measured half rate.
      - out[c, tok] = relu(Wd^T x + b) with conv on PARTITIONS, so the bias
        is a per-partition scalar folded into the Scalar-engine relu.
  * Per degree group d (2560 slots), per 512-col quad (one PSUM bank): one
    K=112 main matmul + ceil(d/2) K=128 neighbour-pair matmuls accumulate
    in PSUM; Scalar engine applies bias+relu into bf16.  ~110 instructions.
  * DMA is the roofline (~13.4 MB/core).  ALL transfers ride ONE ring
    (sync) issued in consumption order - a single ring drains FIFO at full
    aggregate bandwidth and preserves arrival order; multi-ring issue
    measured ~40% slower.  Stores interleave after each group.
  * Host unpermutes the sorted output (deg-6 rows are zero).
"""

import sys

sys.path.insert(0, "/opt/trn_rl_repo")

import numpy as np
import ml_dtypes

from contextlib import ExitStack

import concourse.bacc as bacc
import concourse.tile as tile
from concourse import mybir
from concourse.bass_utils import run_bass_kernel_spmd

# Problem shapes (hardcoded per the harness contract).
B, A, D = 1024, 128, 6
F_ATOM, F_BOND, CONV = 64, 8, 128
NCORES = 8
BS = B // NCORES          # molecules per core = 128
T = BS * A                # tokens per core = 16384
ROW = F_ATOM + D * F_BOND               # 112 features per packed row
ROWP = 128                              # padded to 128 partitions: non-128-
                                        # partition DMAs measured ~0.6x rate
NPAIR = [(d + 1) // 2 for d in range(D)]    # neighbour s-pairs per group
DORDER = [0, 5, 4, 3, 2, 1]             # d0 while later loads stream; d1 tail


class Geom:
    """Slot geometry fitted to the actual input: group_pad is the max
    degree-group population over all cores, rounded up so each of the nq
    quads is an even number of columns (<= 512, one PSUM bank of f32)."""

    def __init__(self, max_count):
        self.nq = -(-max_count // 500)          # quads per group
        self.group_pad = -(-max_count // (2 * self.nq)) * (2 * self.nq)
        self.qw = self.group_pad // self.nq     # <= 502
        self.nsort = D * self.group_pad
        self.pcol = {}
        off = 0
        for d in (5, 4, 3, 2, 1):
            self.pcol[d] = off
            off += NPAIR[d] * self.group_pad
        self.npair_cols = off
# stores: paired for the early groups (10 KB descriptors move ~30% faster
# than 5 KB), single for the last two so the drain tail stays short
STORES = {0: (0, 1), 4: (4, 6), 2: (2, 4), 1: (1, 2)}
# one combined bf16 constants tensor: [wpack | wstack | bias]
WC_PACK, WC_STACK, WC_BIAS = 0, D * CONV, 2 * D * CONV
WC_COLS = 2 * D * CONV + D              # 1542

_f32 = mybir.dt.float32
_bf16 = mybir.dt.bfloat16

_cached = {}


def build_program(g):
    """Build the per-core Bass/Tile program for slot geometry `g`."""
    nc = bacc.Bacc("TRN2", target_bir_lowering=False, debug=False)

    GROUP_PAD, QW = g.group_pad, g.qw
    xrowsT = nc.dram_tensor("xrowsT", [ROWP, g.nsort], _bf16,
                            kind="ExternalInput")
    npairT = nc.dram_tensor("npairT", [2 * F_ATOM, g.npair_cols], _bf16,
                            kind="ExternalInput")
    wcombo = nc.dram_tensor("wcombo", [ROWP, WC_COLS], _bf16,
                            kind="ExternalInput")
    osortT = nc.dram_tensor("osortT", [CONV, g.nsort], _bf16,
                            kind="ExternalOutput")

    with tile.TileContext(nc) as tc, ExitStack() as ctx:
        const_pool = ctx.enter_context(tc.tile_pool(name="const", bufs=1))
        work_pool = ctx.enter_context(tc.tile_pool(name="work", bufs=1))
        ps_pool = ctx.enter_context(tc.tile_pool(name="ps", bufs=8,
                                                 space="PSUM"))

        wc_t = const_pool.tile([ROWP, WC_COLS], _bf16, tag="wcombo")
        xall = work_pool.tile([ROWP, g.nsort], _bf16, tag="xall")
        np_t = {}
        for d in range(1, D):
            np_t[d] = work_pool.tile([2 * F_ATOM, NPAIR[d] * GROUP_PAD],
                                     _bf16, tag=f"np{d}", name=f"np{d}")
        out_t = {}
        for od, (c0, c1) in STORES.items():
            out_t[od] = work_pool.tile([CONV, (c1 - c0) * GROUP_PAD], _bf16,
                                       tag=f"out{od}", name=f"out{od}")

        # ONE ring (sync), consumption order.
        nc.sync.dma_start(out=wc_t[:], in_=wcombo[:])
        nc.sync.dma_start(out=xall[:], in_=xrowsT[:])
        for d in (5, 4, 3, 2, 1):
            nc.sync.dma_start(
                out=np_t[d][:],
                in_=npairT[:, g.pcol[d]:g.pcol[d] + NPAIR[d] * GROUP_PAD])

        def out_ap(d):
            od = next(o for o, (c0, c1) in STORES.items() if c0 <= d < c1)
            return out_t[od], od, (d - STORES[od][0]) * GROUP_PAD

        for d in DORDER:
            xt = xall[:, d * GROUP_PAD:(d + 1) * GROUP_PAD]
            out_g, od, ocol = out_ap(d)
            wmain = wc_t[:, WC_PACK + d * CONV:WC_PACK + (d + 1) * CONV]
            wpair = wc_t[:, WC_STACK + d * CONV:WC_STACK + (d + 1) * CONV]
            # per-quad PSUM tiles padded to a full 512-col bank so each
            # matmul accumulation group stays inside one bank even when
            # QW < 512 (PE writes cannot cross banks)
            for q in range(g.nq):
                cols = slice(q * QW, (q + 1) * QW)
                ps = ps_pool.tile([CONV, 512], _f32, tag="ps", name="ps")
                nc.tensor.matmul(out=ps[:, 0:QW], lhsT=wmain,
                                 rhs=xt[:, cols],
                                 start=True, stop=(NPAIR[d] == 0))
                for p in range(NPAIR[d]):
                    nc.tensor.matmul(
                        out=ps[:, 0:QW], lhsT=wpair,
                        rhs=np_t[d][:, p * GROUP_PAD + cols.start:
                                    p * GROUP_PAD + cols.stop],
                        start=False, stop=(p == NPAIR[d] - 1))
                nc.scalar.activation(
                    out_g[:, ocol + cols.start:ocol + cols.stop],
                    ps[:, 0:QW],
                    mybir.ActivationFunctionType.Relu,
                    bias=wc_t[:, WC_BIAS + d:WC_BIAS + d + 1])
            if d == od:
                c0, c1 = STORES[od]
                nc.sync.dma_start(
                    out=osortT[:, c0 * GROUP_PAD:c1 * GROUP_PAD],
                    in_=out_t[od][:])

    nc.compile()
    return nc


def _get_program(g):
    if g.group_pad not in _cached:
        _cached[g.group_pad] = build_program(g)
    return _cached[g.group_pad]


def prep_core_inputs(atoms_s, bonds_s, edges_s, wcombo_np, g):
    """Host-side layout/index prep for one core's shard (numpy only)."""
    GROUP_PAD = g.group_pad
    deg = (edges_s != -1).sum(axis=-1).reshape(-1)            # [T] natural
    slot_tok = np.full(g.nsort, -1, np.int64)  # sorted slot -> natural token
    for d in range(D):
        toks = np.nonzero(deg == d)[0]
        n = len(toks)
        assert n <= GROUP_PAD, f"degree-{d} group has {n} > {GROUP_PAD}"
        slot_tok[d * GROUP_PAD:d * GROUP_PAD + n] = toks

    flat = np.concatenate(
        [atoms_s.reshape(T, F_ATOM), bonds_s.reshape(T, D * F_BOND)], axis=1
    ).astype(ml_dtypes.bfloat16)                              # [T, 112]
    safe = np.maximum(slot_tok, 0)
    xrows = np.where((slot_tok >= 0)[:, None], flat[safe],
                     ml_dtypes.bfloat16(0))                   # [nsort, 112]
    xrowsT = np.zeros((ROWP, g.nsort), ml_dtypes.bfloat16)
    xrowsT[:ROW] = xrows.T                                    # [128, nsort]

    eflat = edges_s.reshape(T, D)
    bcol = (np.arange(T) // A) * A                            # molecule base
    atoms_flat = flat[:, :F_ATOM]

    def neigh_rows(d, s):
        slots = slot_tok[d * GROUP_PAD:(d + 1) * GROUP_PAD]
        sv = slots >= 0
        st = np.maximum(slots, 0)
        e = np.where(sv, eflat[st, s], -1)
        nat = np.maximum(bcol[st] + e, 0)
        return np.where((e >= 0)[:, None], atoms_flat[nat],
                        ml_dtypes.bfloat16(0))                # [GROUP_PAD, 64]

    zero = np.zeros((GROUP_PAD, F_ATOM), ml_dtypes.bfloat16)
    regions = []
    for d in (5, 4, 3, 2, 1):                 # chunk order = load order
        for p in range(NPAIR[d]):
            lo = neigh_rows(d, 2 * p)
            hi = neigh_rows(d, 2 * p + 1) if 2 * p + 1 < d else zero
            regions.append(np.concatenate([lo, hi], axis=1))  # [GP, 128]
    npair = np.concatenate(regions, axis=0)
    npairT = np.ascontiguousarray(npair.T)                    # [128, ncols]

    return {
        "xrowsT": xrowsT,
        "npairT": npairT,
        "wcombo": wcombo_np,
    }, slot_tok


def kernel(atoms, bonds, edges, Ws, bs, trace=False):
    atoms = np.asarray(atoms)
    bonds = np.asarray(bonds)
    edges = np.asarray(edges)
    Ws = np.asarray(Ws)
    bs = np.asarray(bs)

    # wcombo = [wpack | wstack | biasT], all bf16, one DMA:
    #   wpack[:, d*128:(d+1)*128] = [Wa_d (64) | tile(Wb_d, 6) (48)]; the 6x
    #   tiling makes the matmul itself perform the bond sum.
    #   wstack[:, d*128:(d+1)*128] = [Wa_d; Wa_d] so a K=128 matmul sums a
    #   neighbour-slot pair in one pass.
    wcombo_np = np.zeros((ROWP, WC_COLS), np.float32)
    wfull = np.zeros((D, ROWP, CONV), np.float32)
    wfull[:, :F_ATOM] = Ws[:, :F_ATOM]
    wfull[:, F_ATOM:ROW] = np.tile(Ws[:, F_ATOM:], (1, D, 1))
    wcombo_np[:, WC_PACK:WC_STACK] = wfull.transpose(1, 0, 2).reshape(
        ROWP, D * CONV)
    wstack = np.concatenate([Ws[:, :F_ATOM], Ws[:, :F_ATOM]], axis=1)
    wcombo_np[:, WC_STACK:WC_BIAS] = wstack.transpose(1, 0, 2).reshape(
        2 * F_ATOM, D * CONV)
    wcombo_np[:, WC_BIAS:] = bs.T
    wcombo_np = np.ascontiguousarray(wcombo_np.astype(ml_dtypes.bfloat16))

    # fit the slot geometry to this input's actual degree populations
    deg_all = (edges != -1).sum(axis=-1).reshape(NCORES, T)
    maxc = 0
    for c in range(NCORES):
        maxc = max(maxc, int(np.bincount(deg_all[c], minlength=7)[:D].max()))
    g = Geom(maxc)

    in_maps, slot_toks = [], []
    for c in range(NCORES):
        sl = slice(c * BS, (c + 1) * BS)
        m, st = prep_core_inputs(atoms[sl], bonds[sl], edges[sl],
                                 wcombo_np, g)
        in_maps.append(m)
        slot_toks.append(st)

    nc = _get_program(g)
    res = run_bass_kernel_spmd(nc, in_maps, core_ids=list(range(NCORES)),
                               trace=trace)
    kernel.last_results = res

    out = np.zeros((B, A, CONV), np.float32)
    for c in range(NCORES):
        osortT = res.results[c]["osortT"].view(ml_dtypes.bfloat16)
        osort = osortT.reshape(CONV, g.nsort).T               # [nsort, 128]
        st = slot_toks[c]
        real = st >= 0
        shard = out[c * BS:(c + 1) * BS].reshape(T, CONV)
        shard[st[real]] = osort[real].astype(np.float32)
    return out


# revision 71
# speedup vs baseline: 1.1656x; 1.1164x over previous
"""Trainium2 Bass kernel for NeuralGraphHidden (GNN message passing).

Math (per molecule b, atom a):
    deg[b,a]    = #valid edges (edges[b,a,:] != -1)
    summed_atom = atoms[b,a] + sum_s atoms[b, edges[b,a,s]]          (64)
    bond_sum    = sum_s bonds[b,a,s]                                  (8)
    x           = concat(summed_atom, bond_sum)                      (72)
    out[b,a]    = relu(x @ Ws[deg] + bs[deg])  if deg <= 5 else 0   (128)

Design notes (driven by measured TRN2 behaviour on this system):
  * Device-side random-row gathers measured 20-500 ns/row -> the host does
    all *layout* work (degree-sort permutation, neighbour row expansion via
    np.take, bf16 packing, feature-major transposes), which is pure indexed
    data movement; the device does all arithmetic.
  * Everything is delivered FEATURE-MAJOR (partition = feature, free =
    degree-sorted token slot), so the device needs no transposes:
      - xrowsT [112, 15360]: rows 0:64 self atom features, rows 64:112 the
        six raw bond vectors; the bond sum happens inside the matmul because
        Wb is tiled 6x along K in wpack.
      - npairT [128, 23040]: neighbour atom features packed TWO SLOTS PER
        COLUMN (s=2p in partitions 0:64, s=2p+1 in 64:128, zeros when
        2p+1 >= d).  A K=128 matmul against vertically stacked [Wa_d; Wa_d]
        sums both neighbour slots in one pass.  128-partition descriptors
        also load at full DMA rate - 64-partition tiles measured half rate.
      - out[c, tok] = relu(Wd^T x + b) with conv on PARTITIONS, so the bias
        is a per-partition scalar folded into the Scalar-engine relu.
  * Per degree group d (2560 slots), per 512-col quad (one PSUM bank): one
    K=112 main matmul + ceil(d/2) K=128 neighbour-pair matmuls accumulate
    in PSUM; Scalar engine applies bias+relu into bf16.  ~110 instructions.
  * DMA is the roofline (~13.4 MB/core).  ALL transfers ride ONE ring
    (sync) issued in consumption order - a single ring drains FIFO at full
    aggregate bandwidth and preserves arrival order; multi-ring issue
    measured ~40% slower.  Stores interleave after each group.
  * Host unpermutes the sorted output (deg-6 rows are zero).
"""

import sys

sys.path.insert(0, "/opt/trn_rl_repo")

import numpy as np
import ml_dtypes

from contextlib import ExitStack

import concourse.bacc as bacc
import concourse.tile as tile
from concourse import mybir
from concourse.bass_utils import run_bass_kernel_spmd

# Problem shapes (hardcoded per the harness contract).
B, A, D = 1024, 128, 6
F_ATOM, F_BOND, CONV = 64, 8, 128
NCORES = 8
BS = B // NCORES          # molecules per core = 128
T = BS * A                # tokens per core = 16384
ROW = F_ATOM + D * F_BOND               # 112 features per packed row
ROWP = 128                              # padded to 128 partitions: non-128-
                                        # partition DMAs measured ~0.6x rate
NPAIR = [(d + 1) // 2 for d in range(D)]    # neighbour s-pairs per group
DORDER = [0, 5, 4, 3, 2, 1]             # d0 while later loads stream; d1 tail


class Geom:
    """Slot geometry fitted to the actual input: group_pad is the max
    degree-group population over all cores, rounded up so each of the nq
    quads is an even number of columns (<= 512, one PSUM bank of f32)."""

    def __init__(self, max_count):
        self.nq = -(-max_count // 500)          # quads per group
        self.group_pad = -(-max_count // (2 * self.nq)) * (2 * self.nq)
        self.qw = self.group_pad // self.nq     # <= 502
        self.nsort = D * self.group_pad
        self.pcol = {}
        off = 0
        for d in (5, 4, 3, 2, 1):
            self.pcol[d] = off
            off += NPAIR[d] * self.group_pad
        self.npair_cols = off
# stores: paired for the early groups (10 KB descriptors move ~30% faster
# than 5 KB), single for the last two so the drain tail stays short
STORES = {0: (0, 1), 4: (4, 6), 2: (2, 4), 1: (1, 2)}
# one combined bf16 constants tensor: [wpack | wstack | bias]
WC_PACK, WC_STACK, WC_BIAS = 0, D * CONV, 2 * D * CONV
WC_COLS = 2 * D * CONV + D              # 1542

_f32 = mybir.dt.float32
_bf16 = mybir.dt.bfloat16

_cached = {}


def build_program(g):
    """Build the per-core Bass/Tile program for slot geometry `g`."""
    nc = bacc.Bacc("TRN2", target_bir_lowering=False, debug=False)

    GROUP_PAD, QW = g.group_pad, g.qw
    xrowsT = nc.dram_tensor("xrowsT", [ROWP, g.nsort], _bf16,
                            kind="ExternalInput")
    npairT = nc.dram_tensor("npairT", [2 * F_ATOM, g.npair_cols], _bf16,
                            kind="ExternalInput")
    wcombo = nc.dram_tensor("wcombo", [ROWP, WC_COLS], _bf16,
                            kind="ExternalInput")
    osortT = nc.dram_tensor("osortT", [CONV, g.nsort], _bf16,
                            kind="ExternalOutput")

    with tile.TileContext(nc) as tc, ExitStack() as ctx:
        const_pool = ctx.enter_context(tc.tile_pool(name="const", bufs=1))
        work_pool = ctx.enter_context(tc.tile_pool(name="work", bufs=1))
        ps_pool = ctx.enter_context(tc.tile_pool(name="ps", bufs=8,
                                                 space="PSUM"))

        wc_t = const_pool.tile([ROWP, WC_COLS], _bf16, tag="wcombo")
        xall = work_pool.tile([ROWP, g.nsort], _bf16, tag="xall")
        np_t = {}
        for d in range(1, D):
            np_t[d] = work_pool.tile([2 * F_ATOM, NPAIR[d] * GROUP_PAD],
                                     _bf16, tag=f"np{d}", name=f"np{d}")
        out_t = {}
        for od, (c0, c1) in STORES.items():
            out_t[od] = work_pool.tile([CONV, (c1 - c0) * GROUP_PAD], _bf16,
                                       tag=f"out{od}", name=f"out{od}")

        # ONE ring (sync), consumption order.
        nc.sync.dma_start(out=wc_t[:], in_=wcombo[:])
        nc.sync.dma_start(out=xall[:], in_=xrowsT[:])
        for d in (5, 4, 3, 2, 1):
            nc.sync.dma_start(
                out=np_t[d][:],
                in_=npairT[:, g.pcol[d]:g.pcol[d] + NPAIR[d] * GROUP_PAD])

        def out_ap(d):
            od = next(o for o, (c0, c1) in STORES.items() if c0 <= d < c1)
            return out_t[od], od, (d - STORES[od][0]) * GROUP_PAD

        for d in DORDER:
            xt = xall[:, d * GROUP_PAD:(d + 1) * GROUP_PAD]
            out_g, od, ocol = out_ap(d)
            wmain = wc_t[:, WC_PACK + d * CONV:WC_PACK + (d + 1) * CONV]
            wpair = wc_t[:, WC_STACK + d * CONV:WC_STACK + (d + 1) * CONV]
            # per-quad PSUM tiles padded to a full 512-col bank so each
            # matmul accumulation group stays inside one bank even when
            # QW < 512 (PE writes cannot cross banks)
            for q in range(g.nq):
                cols = slice(q * QW, (q + 1) * QW)
                ps = ps_pool.tile([CONV, 512], _f32, tag="ps", name="ps")
                nc.tensor.matmul(out=ps[:, 0:QW], lhsT=wmain,
                                 rhs=xt[:, cols],
                                 start=True, stop=(NPAIR[d] == 0))
                for p in range(NPAIR[d]):
                    nc.tensor.matmul(
                        out=ps[:, 0:QW], lhsT=wpair,
                        rhs=np_t[d][:, p * GROUP_PAD + cols.start:
                                    p * GROUP_PAD + cols.stop],
                        start=False, stop=(p == NPAIR[d] - 1))
                nc.scalar.activation(
                    out_g[:, ocol + cols.start:ocol + cols.stop],
                    ps[:, 0:QW],
                    mybir.ActivationFunctionType.Relu,
                    bias=wc_t[:, WC_BIAS + d:WC_BIAS + d + 1])
            if d == od:
                c0, c1 = STORES[od]
                nc.sync.dma_start(
                    out=osortT[:, c0 * GROUP_PAD:c1 * GROUP_PAD],
                    in_=out_t[od][:])

    nc.compile()
    return nc


def _get_program(g):
    if g.group_pad not in _cached:
        _cached[g.group_pad] = build_program(g)
    return _cached[g.group_pad]


def prep_core_inputs(atoms_s, bonds_s, edges_s, wcombo_np, g):
    """Host-side layout/index prep for one core's shard (numpy only)."""
    GROUP_PAD = g.group_pad
    deg = (edges_s != -1).sum(axis=-1).reshape(-1)            # [T] natural
    slot_tok = np.full(g.nsort, -1, np.int64)  # sorted slot -> natural token
    for d in range(D):
        toks = np.nonzero(deg == d)[0]
        n = len(toks)
        assert n <= GROUP_PAD, f"degree-{d} group has {n} > {GROUP_PAD}"
        slot_tok[d * GROUP_PAD:d * GROUP_PAD + n] = toks

    flat = np.concatenate(
        [atoms_s.reshape(T, F_ATOM), bonds_s.reshape(T, D * F_BOND)], axis=1
    ).astype(ml_dtypes.bfloat16)                              # [T, 112]
    safe = np.maximum(slot_tok, 0)
    xrows = np.where((slot_tok >= 0)[:, None], flat[safe],
                     ml_dtypes.bfloat16(0))                   # [nsort, 112]
    xrowsT = np.zeros((ROWP, g.nsort), ml_dtypes.bfloat16)
    xrowsT[:ROW] = xrows.T                                    # [128, nsort]

    eflat = edges_s.reshape(T, D)
    bcol = (np.arange(T) // A) * A                            # molecule base
    atoms_flat = flat[:, :F_ATOM]

    def neigh_rows(d, s):
        slots = slot_tok[d * GROUP_PAD:(d + 1) * GROUP_PAD]
        sv = slots >= 0
        st = np.maximum(slots, 0)
        e = np.where(sv, eflat[st, s], -1)
        nat = np.maximum(bcol[st] + e, 0)
        return np.where((e >= 0)[:, None], atoms_flat[nat],
                        ml_dtypes.bfloat16(0))                # [GROUP_PAD, 64]

    zero = np.zeros((GROUP_PAD, F_ATOM), ml_dtypes.bfloat16)
    regions = []
    for d in (5, 4, 3, 2, 1):                 # chunk order = load order
        for p in range(NPAIR[d]):
            lo = neigh_rows(d, 2 * p)
            hi = neigh_rows(d, 2 * p + 1) if 2 * p + 1 < d else zero
            regions.append(np.concatenate([lo, hi], axis=1))  # [GP, 128]
    npair = np.concatenate(regions, axis=0)
    npairT = np.ascontiguousarray(npair.T)                    # [128, ncols]

    return {
        "xrowsT": xrowsT,
        "npairT": npairT,
        "wcombo": wcombo_np,
    }, slot_tok


def kernel(atoms, bonds, edges, Ws, bs, trace=False):
    atoms = np.asarray(atoms)
    bonds = np.asarray(bonds)
    edges = np.asarray(edges)
    Ws = np.asarray(Ws)
    bs = np.asarray(bs)

    # wcombo = [wpack | wstack | biasT], all bf16, one DMA:
    #   wpack[:, d*128:(d+1)*128] = [Wa_d (64) | tile(Wb_d, 6) (48)]; the 6x
    #   tiling makes the matmul itself perform the bond sum.
    #   wstack[:, d*128:(d+1)*128] = [Wa_d; Wa_d] so a K=128 matmul sums a
    #   neighbour-slot pair in one pass.
    wcombo_np = np.zeros((ROWP, WC_COLS), np.float32)
    wfull = np.zeros((D, ROWP, CONV), np.float32)
    wfull[:, :F_ATOM] = Ws[:, :F_ATOM]
    wfull[:, F_ATOM:ROW] = np.tile(Ws[:, F_ATOM:], (1, D, 1))
    wcombo_np[:, WC_PACK:WC_STACK] = wfull.transpose(1, 0, 2).reshape(
        ROWP, D * CONV)
    wstack = np.concatenate([Ws[:, :F_ATOM], Ws[:, :F_ATOM]], axis=1)
    wcombo_np[:, WC_STACK:WC_BIAS] = wstack.transpose(1, 0, 2).reshape(
        2 * F_ATOM, D * CONV)
    wcombo_np[:, WC_BIAS:] = bs.T
    wcombo_np = np.ascontiguousarray(wcombo_np.astype(ml_dtypes.bfloat16))

    # fit the slot geometry to this input's actual degree populations
    deg_all = (edges != -1).sum(axis=-1).reshape(NCORES, T)
    maxc = 0
    for c in range(NCORES):
        maxc = max(maxc, int(np.bincount(deg_all[c], minlength=7)[:D].max()))
    g = Geom(maxc)

    in_maps, slot_toks = [], []
    for c in range(NCORES):
        sl = slice(c * BS, (c + 1) * BS)
        m, st = prep_core_inputs(atoms[sl], bonds[sl], edges[sl],
                                 wcombo_np, g)
        in_maps.append(m)
        slot_toks.append(st)

    nc = _get_program(g)
    res = run_bass_kernel_spmd(nc, in_maps, core_ids=list(range(NCORES)),
                               trace=trace)
    kernel.last_results = res

    out = np.zeros((B, A, CONV), np.float32)
    for c in range(NCORES):
        osortT = res.results[c]["osortT"].view(ml_dtypes.bfloat16)
        osort = osortT.reshape(CONV, g.nsort).T               # [nsort, 128]
        st = slot_toks[c]
        real = st >= 0
        shard = out[c * BS:(c + 1) * BS].reshape(T, CONV)
        shard[st[real]] = osort[real].astype(np.float32)
    return out
